# revision 23
# baseline (speedup 1.0000x reference)
"""Trainium2 Bass kernel for nn_AttentionHead (B=8, T=2048, D=1024, H=64).

Single attention head with additive relative-position scores:
    k = x@Wk + bk; q = x@Wq + bq; v = x@Wv
    S = (q k^T) sqrt(H) + einsum(btc,tvc->btv)(q, rel)  [+ causal mask]
    out = softmax(S) @ v

v2 design (evidence-driven rewrite of the v1 baseline):

Sharding: mod-8 interleaved query rows. Core c owns rows {t : t % 8 == c},
as two 128-row tiles (tile0: t = c+8j, j<128, causal extent 1024; tile1:
t = 1024+c+8j, extent 2048). This makes per-group causal extents UNIFORM
across cores (SPMD) and exact: rel group g covers 8 rows with extent
64*(g%16+1) (+1024 for tile1), so the rel stream moves 34.6MB/core instead
of 50.3MB padded.

Numerics: fp16 everywhere at full PE rate. Projections are 3-pass fp16
hi/lo (x split hi/lo on host, W split hi/lo on host) -> f32 PSUM (error
~1e-7 relative). q/k split on-chip to bf16 hi/lo for the 2-stream qk score
matmul (qh*kh; ql*kh + qh*kl via stacked operands). rel scores: q as fp16
single x rel fp16 single stream (half of v1's hi/lo). Rel scores staged
fp16 with the causal mask folded in; combined with qk scores by an
identity-matmul ADD into the qk PSUM accumulation (no DVE add pass).
Simulated end-to-end numerics: 2.1e-3 rel err (gate 2e-2).

DMA routing (the big v1 bug): scalar(ACT)-issued DMAs land on SDMA
engines 0/1 ONLY (~16GB/s) and gated v1's whole rel phase. v2 issues all
bulk DMA from sync(SP, spreads to 16 engines), scatters from
gpsimd(SWDGE, spreads), gathers from vector. The scalar engine does
compute only (PSUM->SBUF copies, exp).

Collectives (k/v AllGather) are issued mid-stream and overlap the rel
stream; the k/v projection is batch-sharded as in v1.
"""

import os
from contextlib import ExitStack

import numpy as np
import ml_dtypes

import concourse.bass as bass
import concourse.tile as tile
from concourse import bacc, mybir
from concourse.bass_utils import run_bass_kernel_spmd

BF16 = mybir.dt.bfloat16
F16 = mybir.dt.float16
F32 = mybir.dt.float32

B, T, D, H = 8, 2048, 1024, 64
TB = 128
NCORES = 8
NEG = -240.0
ND = D // 128              # 8 d-tiles
SCHP = 256                 # proj chunk cols
NGRP = 32                  # rel groups (16 per tile), 8 rows each
SCH = 512                  # rel/qk v-chunk

LAST_EXEC_NS = None
LAST_RES = None
DEBUG = os.environ.get("KDEBUG") == "1"


def _install_ntff_hook():
    import sys
    import types
    try:
        import antenv.axon_hooks  # noqa: F401
        return
    except ImportError:
        pass
    try:
        import antenv
        from trn_agent_boot.trn_boot import _ntff_profile_via_ctypes
        hook = _ntff_profile_via_ctypes("/opt/axon/libaxon_pjrt.so")
        mod = types.ModuleType("antenv.axon_hooks")
        mod._hook = hook
        mod.get_axon_ntff_profile_hook = lambda: mod._hook

        def _set(h):
            mod._hook = h

        mod.set_axon_ntff_profile_hook = _set
        antenv.axon_hooks = mod
        sys.modules["antenv.axon_hooks"] = mod
    except Exception:
        pass


def group_ext(g: int, causal: bool) -> int:
    """Causal rel extent (cols) for group g. Uniform across cores."""
    if not causal:
        return T
    if g < 16:
        return 64 * (g + 1)
    return 1024 + 64 * (g - 16 + 1)


def rel_chunks(g: int, causal: bool):
    """(offset, width) v-chunks for group g (width <= SCH)."""
    ext = group_ext(g, causal)
    out = []
    v0 = 0
    while v0 < ext:
        w = min(SCH, ext - v0)
        out.append((v0, w))
        v0 += w
    return out


def build_nc(causal: bool):
    exts = (1024, 2048) if causal else (2048, 2048)
    NST = T // 128            # v tiles per batch (16)

    nc = bacc.Bacc("TRN2", target_bir_lowering=False, debug=False,
                   num_devices=NCORES)

    # ---- I/O ----
    xh = nc.dram_tensor("xh", [D, T], F16, kind="ExternalInput")
    xl = nc.dram_tensor("xl", [D, T], F16, kind="ExternalInput")
    xqh = nc.dram_tensor("xqh", [D, 2, B, TB], F16, kind="ExternalInput")
    xql = nc.dram_tensor("xql", [D, 2, B, TB], F16, kind="ExternalInput")
    wkh = nc.dram_tensor("wkh", [D, H], F16, kind="ExternalInput")
    wkl = nc.dram_tensor("wkl", [D, H], F16, kind="ExternalInput")
    wqh = nc.dram_tensor("wqh", [D, H], F16, kind="ExternalInput")
    wql = nc.dram_tensor("wql", [D, H], F16, kind="ExternalInput")
    wv = nc.dram_tensor("wv", [D, H], F16, kind="ExternalInput")
    bk8 = nc.dram_tensor("bk8", [H, 1], F32, kind="ExternalInput")
    bq_ = nc.dram_tensor("bq", [H, 1], F32, kind="ExternalInput")
    # rel tiles, flattened stream: per (g, chunk) block [128, 4*w]
    totcols = sum(4 * w for g in range(NGRP) for _, w in rel_chunks(g, causal))
    relh = nc.dram_tensor("relh", [128, totcols], F16, kind="ExternalInput")
    maskrel = nc.dram_tensor("maskrel", [NGRP, 128, 64], F16,
                             kind="ExternalInput")
    identb = nc.dram_tensor("identb", [128, 128], F16, kind="ExternalInput")
    identf = nc.dram_tensor("identf", [64, 64], F16, kind="ExternalInput")
    out = nc.dram_tensor("out", [B, 2, TB, H], F32, kind="ExternalOutput")

    if DEBUG:
        dS0 = nc.dram_tensor("dS0", [TB, B * exts[0]], F16,
                             kind="ExternalOutput")
        dS1 = nc.dram_tensor("dS1", [TB, B * exts[1]], F16,
                             kind="ExternalOutput")
        dq16 = nc.dram_tensor("dq16", [64, 2 * B * TB], F16,
                              kind="ExternalOutput")
        dkst = nc.dram_tensor("dkst", [128, B * T], BF16,
                              kind="ExternalOutput")
        dvn = nc.dram_tensor("dvn", [128, B * (T // 128) * H], F16,
                             kind="ExternalOutput")
        dkgin = nc.dram_tensor("dkgin", [128, T], BF16,
                               kind="ExternalOutput")
        dkgout = nc.dram_tensor("dkgout", [NCORES, 128, T], BF16,
                                kind="ExternalOutput")
        dS2b0 = nc.dram_tensor("dS2b0", [TB, 1024], F32,
                               kind="ExternalOutput")
        dPb0 = nc.dram_tensor("dPb0", [TB, 1024], F16,
                              kind="ExternalOutput")
        dstat = nc.dram_tensor("dstat", [TB, 8], F32, kind="ExternalOutput")

    kg_in = nc.dram_tensor("kg_in", [128, T], BF16)
    kg_out = nc.dram_tensor("kg_out", [NCORES, 128, T], BF16,
                            addr_space="Shared")
    vg_in = nc.dram_tensor("vg_in", [128, NST * H], F16)
    vg_out = nc.dram_tensor("vg_out", [NCORES, 128, NST * H], F16,
                            addr_space="Shared")

    with tile.TileContext(nc) as tc:
        with (
            tc.tile_pool(name="persist", bufs=1) as pp,
            tc.tile_pool(name="weights", bufs=1) as pw,
            tc.tile_pool(name="Spool", bufs=1) as pS,
            tc.tile_pool(name="kv", bufs=1) as pkv,
            tc.tile_pool(name="relstream", bufs=2) as prel,
            tc.tile_pool(name="stg", bufs=3) as pstg,
            tc.tile_pool(name="stats", bufs=4) as pstat,
            tc.tile_pool(name="psrel", bufs=2, space="PSUM") as ppr,
        ):
            # persistent q tiles: cols (blk, b, t_local)
            qhi = pp.tile([64, 2 * B * TB], BF16, tag="qhi")
            qc = pp.tile([128, 2 * B * TB], BF16, tag="qc")   # [lo;hi]
            q16 = pp.tile([64, 2 * B * TB], F16, tag="q16")
            bd0 = pp.tile([128, (TB // 2) * 16], F16, tag="bd0")
            bd1 = pp.tile([128, (TB // 2) * 16], F16, tag="bd1")
            mrel = pp.tile([128, NGRP, 64], F16, tag="mrel")
            idb = pw.tile([128, 128], F16, tag="identb")
            idf = pw.tile([64, 64], F16, tag="identf")
            wk_t = pw.tile([128, ND, 2, H], F16, tag="wk")
            wq_t = pw.tile([128, ND, 2, H], F16, tag="wq")
            wv_t = pw.tile([128, ND, H], F16, tag="wv")
            bk_t = pw.tile([H, 1], F32, tag="bk")
            bq_t = pw.tile([H, 1], F32, tag="bq")

            nc.sync.dma_start(idb, identb.ap())
            nc.sync.dma_start(idf, identf.ap())
            nc.sync.dma_start(mrel, maskrel.ap().rearrange("g p v -> p g v"))
            nc.sync.dma_start(
                wk_t[:, :, 0, :], wkh.ap().rearrange("(n p) h -> p n h", p=128))
            nc.sync.dma_start(
                wk_t[:, :, 1, :], wkl.ap().rearrange("(n p) h -> p n h", p=128))
            nc.sync.dma_start(
                wq_t[:, :, 0, :], wqh.ap().rearrange("(n p) h -> p n h", p=128))
            nc.sync.dma_start(
                wq_t[:, :, 1, :], wql.ap().rearrange("(n p) h -> p n h", p=128))
            nc.sync.dma_start(
                wv_t, wv.ap().rearrange("(n p) h -> p n h", p=128))
            nc.sync.dma_start(bk_t, bk8.ap())
            nc.sync.dma_start(bq_t, bq_.ap())

            # rel+mask staging target: S tiles per block, fp16, init NEG
            S_all = [pS.tile([TB, B * exts[blk]], F16, tag=f"S{blk}",
                             name=f"S_{blk}")
                     for blk in range(2)]
            if causal:
                nc.vector.memset(S_all[0], NEG)
                nc.vector.memset(S_all[1], NEG)

            kstack = pkv.tile([128, B * T], BF16, tag="kstack")
            vnat = pkv.tile([128, B * NST * H], F16, tag="vnat")

            with ExitStack() as stk:
                ent = stk.enter_context
                pxh = ent(tc.tile_pool(name="xh", bufs=2))
                pxl = ent(tc.tile_pool(name="xl", bufs=2))
                pst = ent(tc.tile_pool(name="ptmp", bufs=2))
                ppush = ent(tc.tile_pool(name="push", bufs=2))
                ppmm = ent(tc.tile_pool(name="psproj", bufs=2, space="PSUM"))
                ppv = ent(tc.tile_pool(name="psv", bufs=1, space="PSUM"))
                ppvt = ent(tc.tile_pool(name="psvt", bufs=1, space="PSUM"))

                def kv_chunk(ci):
                    c0 = ci * SCHP
                    xf = xh.ap().rearrange("(n p) t -> p n t", p=128)
                    xlf = xl.ap().rearrange("(n p) t -> p n t", p=128)
                    xht = pxh.tile([128, ND, SCHP], F16, tag="xh")
                    xlt = pxl.tile([128, ND, SCHP], F16, tag="xl")
                    nc.sync.dma_start(xht, xf[:, :, c0:c0 + SCHP])
                    nc.sync.dma_start(xlt, xlf[:, :, c0:c0 + SCHP])
                    psk = ppmm.tile([H, SCHP], F32, tag="psp")
                    for dt_ in range(ND):
                        nc.tensor.matmul(psk, wk_t[:, dt_, 0, :], xht[:, dt_],
                                         start=(dt_ == 0), stop=False)
                    for dt_ in range(ND):
                        nc.tensor.matmul(psk, wk_t[:, dt_, 1, :], xht[:, dt_],
                                         start=False, stop=False)
                    for dt_ in range(ND):
                        nc.tensor.matmul(psk, wk_t[:, dt_, 0, :], xlt[:, dt_],
                                         start=False, stop=(dt_ == ND - 1))
                    ktmp = pst.tile([H, SCHP], F32, tag="ktmp")
                    nc.scalar.activation(ktmp, psk,
                                         mybir.ActivationFunctionType.Identity,
                                         bias=bk_t[:, :], scale=1.0)
                    kpush = ppush.tile([128, SCHP], BF16, tag="kpush")
                    nc.vector.tensor_copy(kpush[0:H, :], ktmp)
                    nc.vector.tensor_tensor(kpush[H:128, :], ktmp,
                                            kpush[0:H, :],
                                            mybir.AluOpType.subtract)
                    nc.sync.dma_start(kg_in.ap()[:, c0:c0 + SCHP], kpush)
                    psv = ppv.tile([H, SCHP], F32, tag="psv")
                    for dt_ in range(ND):
                        nc.tensor.matmul(psv, wv_t[:, dt_], xht[:, dt_],
                                         start=(dt_ == 0), stop=False)
                    for dt_ in range(ND):
                        nc.tensor.matmul(psv, wv_t[:, dt_], xlt[:, dt_],
                                         start=False, stop=(dt_ == ND - 1))
                    vtmp = pst.tile([H, SCHP], F16, tag="vtmp")
                    nc.scalar.copy(vtmp, psv)
                    vpush = ppush.tile([128, 2 * H], F16, tag="vpush")
                    for sub in range(SCHP // 128):
                        pvt = ppvt.tile([128, H], F16, tag="pvt")
                        nc.tensor.transpose(
                            pvt, vtmp[:, sub * 128:(sub + 1) * 128], idf)
                        nc.vector.tensor_copy(
                            vpush[:, sub * H:(sub + 1) * H], pvt)
                    st0 = c0 // 128
                    nc.sync.dma_start(
                        vg_in.ap()[:, st0 * H:(st0 + 2) * H], vpush)

                def q_chunk(ci):
                    c0 = ci * SCHP
                    xqf = xqh.ap().rearrange("(n p) k b t -> p n (k b t)",
                                             p=128)
                    xqlf = xql.ap().rearrange("(n p) k b t -> p n (k b t)",
                                              p=128)
                    xht = pxh.tile([128, ND, SCHP], F16, tag="xh")
                    xlt = pxl.tile([128, ND, SCHP], F16, tag="xl")
                    nc.sync.dma_start(xht, xqf[:, :, c0:c0 + SCHP])
                    nc.sync.dma_start(xlt, xqlf[:, :, c0:c0 + SCHP])
                    psq = ppmm.tile([H, SCHP], F32, tag="psp")
                    for dt_ in range(ND):
                        nc.tensor.matmul(psq, wq_t[:, dt_, 0, :], xht[:, dt_],
                                         start=(dt_ == 0), stop=False)
                    for dt_ in range(ND):
                        nc.tensor.matmul(psq, wq_t[:, dt_, 1, :], xht[:, dt_],
                                         start=False, stop=False)
                    for dt_ in range(ND):
                        nc.tensor.matmul(psq, wq_t[:, dt_, 0, :], xlt[:, dt_],
                                         start=False, stop=(dt_ == ND - 1))
                    qtmp = pst.tile([H, SCHP], F32, tag="ktmp")
                    nc.scalar.activation(qtmp, psq,
                                         mybir.ActivationFunctionType.Identity,
                                         bias=bq_t[:, :], scale=1.0)
                    cs = slice(c0, c0 + SCHP)
                    nc.vector.tensor_copy(qhi[:, cs], qtmp)
                    nc.vector.tensor_tensor(qc[0:H, cs], qtmp, qhi[:, cs],
                                            mybir.AluOpType.subtract)
                    nc.vector.tensor_copy(qc[H:128, cs], qhi[:, cs])
                    nc.vector.tensor_copy(q16[:, cs], qtmp)

                def build_bd(blk):
                    bdt = bd0 if blk == 0 else bd1
                    nc.vector.memset(bdt, 0.0)
                    qblk = (q16[:, blk * B * TB:(blk + 1) * B * TB]
                            .rearrange("c (b t) -> c b t", b=B))
                    for j in range(2):
                        dst = (bdt[j * H:(j + 1) * H]
                               .rearrange("c (p s) -> c p s", s=16)
                               [:, :, j * 8:j * 8 + 8])
                        src = qblk[:, :, j::2].rearrange("c b p -> c p b")
                        nc.vector.tensor_copy(dst, src)

                # ---- generators ----
                ccs = []
                gathered = {}

                def proj_steps():
                    for ci in range(T // SCHP):
                        kv_chunk(ci)
                        yield
                        q_chunk(ci)
                        if ci == 3:
                            build_bd(0)
                        yield
                    build_bd(1)
                    yield
                    ccs.append(nc.gpsimd.collective_compute(
                        "AllGather", mybir.AluOpType.bypass,
                        replica_groups=[list(range(NCORES))],
                        ins=[kg_in[:].opt()], outs=[kg_out[:].opt()]))
                    ccs.append(nc.gpsimd.collective_compute(
                        "AllGather", mybir.AluOpType.bypass,
                        replica_groups=[list(range(NCORES))],
                        ins=[vg_in[:].opt()], outs=[vg_out[:].opt()]))
                    yield

                def rel_group(g, roff):
                    """Stream one rel group; returns new roff."""
                    blk = 0 if g < 16 else 1
                    gl = g - 16 * blk
                    ext = group_ext(g, causal)
                    bdt = bd0 if blk == 0 else bd1
                    stg = pstg.tile([128, exts[1]], F16, tag="stg",
                                    name=f"stg_{g}")
                    for (v0, w) in rel_chunks(g, causal):
                        rht = prel.tile([128, 4, SCH], F16, tag="rht")
                        nc.sync.dma_start(rht[:, :, 0:w],
                                          relh.ap()[:, roff:roff + 4 * w]
                                          .rearrange("p (u v) -> p u v", u=4))
                        roff += 4 * w
                        psr = ppr.tile([128, SCH], F32, tag="psr")
                        for u in range(4):
                            p = 4 * gl + u
                            nc.tensor.matmul(
                                psr[32 * u:32 * u + 16, 0:w],
                                bdt[:, p * 16:p * 16 + 16],
                                rht[:, u, 0:w],
                                start=True, stop=True,
                                tile_position=(0, 32 * u))
                        nc.scalar.copy(stg[:, v0:v0 + w],
                                              psr[:, 0:w])
                    if causal:
                        nc.gpsimd.tensor_tensor(
                            stg[:, ext - 64:ext], stg[:, ext - 64:ext],
                            mrel[:, g, :], mybir.AluOpType.add)
                    # scatter to S layout, one DMA per u-band:
                    # src [16 parts = (j,b), ext] -> dst [2 parts = j, (b, v)]
                    for u in range(4):
                        dst = (S_all[blk][8 * gl + 2 * u:8 * gl + 2 * u + 2, :]
                               .rearrange("j (b v) -> j b v", b=8)
                               [:, :, 0:ext])
                        src = stg[32 * u:32 * u + 16, 0:ext]
                        nc.gpsimd.dma_start(dst, src)
                    return roff

                def gather_kv():
                    # k/v from the AllGathers into SBUF (gpsimd SWDGE,
                    # spreads across all 16 SDMA engines). The collective's
                    # DRAM output is not dependency-tracked by Tile, so add
                    # explicit completion edges.
                    for b in range(B):
                        g1 = nc.gpsimd.dma_start(
                            kstack[:, b * T:(b + 1) * T], kg_out.ap()[b])
                        g2 = nc.gpsimd.dma_start(
                            vnat[:, b * NST * H:(b + 1) * NST * H],
                            vg_out.ap()[b])
                        gathered[b] = (g1, g2)
                        for gi in (g1, g2):
                            for cc in ccs:
                                bass._add_dep_helper(
                                    gi.ins, cc.ins, sync=True,
                                    reason="gather waits for AllGather")

                def rel_steps():
                    roff = 0
                    for g in range(NGRP):
                        roff = rel_group(g, roff)
                        if g == 19:
                            gather_kv()
                        yield g

                # drive proj + rel tile0 interleaved; rel needs bd0 (after
                # q_chunk 3) so prime proj by 9 steps first.
                pgen = proj_steps()
                rgen = rel_steps()
                done_p = False
                for _ in range(9):
                    done_p = next(pgen, "end") == "end"
                last_g = -1
                while last_g < 15:
                    if not done_p:
                        done_p = next(pgen, "end") == "end"
                    last_g = next(rgen)
                while not done_p:
                    done_p = next(pgen, "end") == "end"
                stk.close()

                # ---- qk/softmax/AV phase, interleaved with rel tile1 ----
                with ExitStack() as stk2:
                    ent2 = stk2.enter_context
                    pS2 = ent2(tc.tile_pool(name="S2", bufs=2))
                    pP = ent2(tc.tile_pool(name="P", bufs=2))
                    pPT = ent2(tc.tile_pool(name="PT", bufs=2))
                    po = ent2(tc.tile_pool(name="outp", bufs=2))
                    ppS = ent2(tc.tile_pool(name="psS", bufs=2, space="PSUM"))
                    pppt = ent2(tc.tile_pool(name="pspt", bufs=2,
                                             space="PSUM"))
                    ppav = ent2(tc.tile_pool(name="psav", bufs=1,
                                             space="PSUM"))

                    def qk_b(blk, b):
                        ext = exts[blk]
                        nch = ext // SCH
                        S2 = pS2.tile([TB, exts[1]], F32, tag="S2")
                        pmax = pstat.tile([TB, 4], F32, tag="pmax")
                        qh_s = qhi[:, (blk * B + b) * TB:
                                   (blk * B + b + 1) * TB]
                        qc_s = qc[:, (blk * B + b) * TB:
                                  (blk * B + b + 1) * TB]
                        for ch in range(nch):
                            s0 = ch * SCH
                            psS = ppS.tile([TB, SCH], F32, tag="psS")
                            nc.tensor.matmul(
                                psS, idb,
                                S_all[blk][:, b * ext + s0:b * ext + s0 + SCH],
                                start=True, stop=False)
                            kc = slice(b * T + s0, b * T + s0 + SCH)
                            mmk = nc.tensor.matmul(psS, qh_s, kstack[0:H, kc],
                                                   start=False, stop=False)
                            nc.tensor.matmul(psS, qc_s, kstack[:, kc],
                                             start=False, stop=True)
                            if ch == 0 and b in gathered:
                                for gi in gathered[b]:
                                    bass._add_dep_helper(
                                        mmk.ins, gi.ins, sync=True,
                                        reason="qk waits for k/v gather")
                            nc.scalar.copy(S2[:, s0:s0 + SCH], psS)
                            nc.vector.tensor_reduce(
                                pmax[:, ch:ch + 1], psS,
                                mybir.AxisListType.X, mybir.AluOpType.max)
                        negmax = pstat.tile([TB, 1], F32, tag="negmax")
                        zsum = pstat.tile([TB, 1], F32, tag="zsum")
                        rz = pstat.tile([TB, 1], F32, tag="rz")
                        nc.vector.tensor_reduce(negmax, pmax[:, 0:nch],
                                                mybir.AxisListType.X,
                                                mybir.AluOpType.max,
                                                negate=True)
                        P = pP.tile([TB, exts[1]], F16, tag="P")
                        nc.scalar.activation(P[:, 0:ext], S2[:, 0:ext],
                                             mybir.ActivationFunctionType.Exp,
                                             bias=negmax[:, :], scale=1.0,
                                             accum_out=zsum[:, :])
                        nc.vector.reciprocal(rz, zsum)
                        pso = ppav.tile([TB, H], F32, tag="pso")
                        for st in range(ext // 128):
                            ppt = pppt.tile([128, 128], F16, tag="ppt")
                            nc.tensor.transpose(
                                ppt, P[:, st * 128:(st + 1) * 128], idb)
                            ptt = pPT.tile([128, 128], F16, tag="ptt")
                            nc.vector.tensor_copy(ptt, ppt)
                            nc.tensor.matmul(
                                pso, ptt,
                                vnat[:, (b * NST + st) * H:
                                     (b * NST + st + 1) * H],
                                start=(st == 0), stop=(st == ext // 128 - 1))
                        osb = po.tile([TB, H], F32, tag="osb")
                        nc.vector.tensor_scalar_mul(osb, pso, rz[:, :])
                        nc.sync.dma_start(out.ap()[b, blk], osb)
                        if DEBUG and blk == 0 and b == 0:
                            nc.sync.dma_start(dS2b0.ap(), S2[:, 0:1024])
                            nc.sync.dma_start(dPb0.ap(), P[:, 0:1024])
                            nc.sync.dma_start(dstat.ap()[:, 0:4], pmax)
                            nc.sync.dma_start(dstat.ap()[:, 4:5], negmax)
                            nc.sync.dma_start(dstat.ap()[:, 5:6], zsum)
                            nc.sync.dma_start(dstat.ap()[:, 6:7], rz)

                    # rel tile1 groups interleaved with qk blk0. The k/v
                    # gather is issued inside rel_steps after group 19, so
                    # qk consumers must only be issued after that point.
                    b0 = 0
                    while True:
                        g = next(rgen, None)
                        if g is None:
                            break
                        if g >= 21 and b0 < B:
                            qk_b(0, b0)
                            b0 += 1
                    while b0 < B:
                        qk_b(0, b0)
                        b0 += 1
                    for b in range(B):
                        qk_b(1, b)

                    if DEBUG:
                        nc.sync.dma_start(dS0.ap(), S_all[0])
                        nc.sync.dma_start(dS1.ap(), S_all[1])
                        nc.sync.dma_start(dq16.ap(), q16)
                        nc.sync.dma_start(dkst.ap(), kstack)
                        nc.sync.dma_start(dvn.ap(), vnat)
                        nc.gpsimd.dma_start(dkgin.ap(), kg_in.ap())
                        nc.gpsimd.dma_start(dkgout.ap(), kg_out.ap())

    nc.compile()
    return nc


def _split16(a):
    hi = np.asarray(a, dtype=np.float32).astype(np.float16)
    lo = (np.asarray(a, dtype=np.float32) - hi.astype(np.float32)).astype(
        np.float16)
    return hi, lo


def _prep_core(c, xTh, xTl, rel16, causal):
    """Per-core input tensors (mod-8 row sharding)."""
    # q-projection x: cols (blk, b, t_local); t = blk*1024 + c + 8*j
    idx0 = c + 8 * np.arange(TB)
    idx1 = 1024 + c + 8 * np.arange(TB)
    xqh_c = np.stack([xTh[:, :, idx0], xTh[:, :, idx1]], axis=1)
    xql_c = np.stack([xTl[:, :, idx0], xTl[:, :, idx1]], axis=1)

    # rel stream: per (g, chunk) blocks [128, 4*w]
    blocks = []
    mtiles = np.zeros((NGRP, 128, 64), dtype=np.float16)
    for g in range(NGRP):
        blk, gl = (0, g) if g < 16 else (1, g - 16)
        base = blk * 1024
        trows = base + c + 8 * (8 * gl + np.arange(8))     # 8 query rows
        ext = group_ext(g, causal)
        A = rel16[trows, :ext, :]                          # [8, ext, 64]
        A = (A.reshape(4, 2, ext, 64).transpose(1, 3, 0, 2)
             .reshape(128, 4, ext))                        # [(j,c),(u,v)]
        for (v0, w) in rel_chunks(g, causal):
            blocks.append(np.ascontiguousarray(
                A[:, :, v0:v0 + w].reshape(128, 4 * w)))
        # mask tile for final 64 cols: partitions 32u+8j+b (x=0 half)
        if causal:
            for u in range(4):
                for j in range(2):
                    t = base + c + 8 * (8 * gl + 2 * u + j)
                    vv = ext - 64 + np.arange(64)
                    m = np.where(vv <= t, 0.0, NEG).astype(np.float16)
                    for b in range(8):
                        mtiles[g, 32 * u + 8 * j + b, :] = m
    relflat = np.concatenate(blocks, axis=1)
    return {
        "xh": np.ascontiguousarray(xTh[:, c, :]),
        "xl": np.ascontiguousarray(xTl[:, c, :]),
        "xqh": np.ascontiguousarray(xqh_c),
        "xql": np.ascontiguousarray(xql_c),
        "relh": relflat,
        "maskrel": mtiles,
    }


def kernel(x, Wk, bk, Wq, bq, Wv, rel_pos_emb, mask, **_unused):
    global LAST_EXEC_NS, LAST_RES
    x = np.asarray(x, dtype=np.float32)
    causal = bool(np.asarray(mask).item())

    scale = np.float32(np.sqrt(H))
    xT = np.ascontiguousarray(x.transpose(2, 0, 1))        # [D, B, T]
    xTh, xTl = _split16(xT)
    wkh, wkl = _split16(np.asarray(Wk, np.float32) * scale)
    wqh, wql = _split16(np.asarray(Wq, np.float32))
    wv16 = np.asarray(Wv, np.float32).astype(np.float16)
    bk8 = (np.asarray(bk, np.float32) * scale).reshape(H, 1)
    bqr = np.asarray(bq, np.float32).reshape(H, 1)
    rel16 = np.asarray(rel_pos_emb, np.float32).astype(np.float16)
    identb = np.eye(128, dtype=np.float16)
    identf = np.eye(64, dtype=np.float16)

    in_maps = []
    for c in range(NCORES):
        m = _prep_core(c, xTh, xTl, rel16, causal)
        m.update({
            "wkh": wkh, "wkl": wkl, "wqh": wqh, "wql": wql, "wv": wv16,
            "bk8": bk8, "bq": bqr, "identb": identb, "identf": identf,
        })
        in_maps.append(m)

    nc = build_nc(causal)
    if os.environ.get("KERNEL_TRACE") == "1":
        _install_ntff_hook()
        import jax
        jax.devices()
        try:
            res = run_bass_kernel_spmd(
                nc, in_maps, core_ids=list(range(NCORES)), trace=True)
        except Exception:
            res = run_bass_kernel_spmd(
                nc, in_maps, core_ids=list(range(NCORES)))
    else:
        res = run_bass_kernel_spmd(nc, in_maps, core_ids=list(range(NCORES)))
    LAST_EXEC_NS = res.exec_time_ns
    LAST_RES = res

    outf = np.empty((B, T, H), dtype=np.float32)
    for c in range(NCORES):
        oc = res.results[c]["out"]          # [B, 2, TB, H]
        idx0 = c + 8 * np.arange(TB)
        idx1 = 1024 + c + 8 * np.arange(TB)
        outf[:, idx0] = oc[:, 0]
        outf[:, idx1] = oc[:, 1]
    return outf


# revision 31
# speedup vs baseline: 1.1612x; 1.1612x over previous
"""Trainium2 Bass kernel for nn_AttentionHead (B=8, T=2048, D=1024, H=64).

Single attention head with additive relative-position scores:
    k = x@Wk + bk; q = x@Wq + bq; v = x@Wv
    S = (q k^T) sqrt(H) + einsum(btc,tvc->btv)(q, rel)  [+ causal mask]
    out = softmax(S) @ v

v2 design (evidence-driven rewrite of the v1 baseline):

Sharding: mod-8 interleaved query rows. Core c owns rows {t : t % 8 == c},
as two 128-row tiles (tile0: t = c+8j, j<128, causal extent 1024; tile1:
t = 1024+c+8j, extent 2048). This makes per-group causal extents UNIFORM
across cores (SPMD) and exact: rel group g covers 8 rows with extent
64*(g%16+1) (+1024 for tile1), so the rel stream moves 34.6MB/core instead
of 50.3MB padded.

Numerics: fp16 everywhere at full PE rate. Projections are 3-pass fp16
hi/lo (x split hi/lo on host, W split hi/lo on host) -> f32 PSUM (error
~1e-7 relative). q/k split on-chip to bf16 hi/lo for the 2-stream qk score
matmul (qh*kh; ql*kh + qh*kl via stacked operands). rel scores: q as fp16
single x rel fp16 single stream (half of v1's hi/lo). Rel scores staged
fp16 with the causal mask folded in; combined with qk scores by an
identity-matmul ADD into the qk PSUM accumulation (no DVE add pass).
Simulated end-to-end numerics: 2.1e-3 rel err (gate 2e-2).

DMA routing (the big v1 bug): scalar(ACT)-issued DMAs land on SDMA
engines 0/1 ONLY (~16GB/s) and gated v1's whole rel phase. v2 issues all
bulk DMA from sync(SP, spreads to 16 engines), scatters from
gpsimd(SWDGE, spreads), gathers from vector. The scalar engine does
compute only (PSUM->SBUF copies, exp).

Collectives (k/v AllGather) are issued mid-stream and overlap the rel
stream; the k/v projection is batch-sharded as in v1.
"""

import os
from contextlib import ExitStack

import numpy as np
import ml_dtypes

import concourse.bass as bass
import concourse.tile as tile
from concourse import bacc, mybir
from concourse.bass_utils import run_bass_kernel_spmd

BF16 = mybir.dt.bfloat16
F16 = mybir.dt.float16
F32 = mybir.dt.float32

B, T, D, H = 8, 2048, 1024, 64
TB = 128
NCORES = 8
NEG = -240.0
ND = D // 128              # 8 d-tiles
SCHP = 256                 # proj chunk cols
NGRP = 32                  # rel groups (16 per tile), 8 rows each
SCH = 512                  # rel/qk v-chunk

LAST_EXEC_NS = None
LAST_RES = None
DEBUG = os.environ.get("KDEBUG") == "1"


def _install_ntff_hook():
    import sys
    import types
    try:
        import antenv.axon_hooks  # noqa: F401
        return
    except ImportError:
        pass
    try:
        import antenv
        from trn_agent_boot.trn_boot import _ntff_profile_via_ctypes
        hook = _ntff_profile_via_ctypes("/opt/axon/libaxon_pjrt.so")
        mod = types.ModuleType("antenv.axon_hooks")
        mod._hook = hook
        mod.get_axon_ntff_profile_hook = lambda: mod._hook

        def _set(h):
            mod._hook = h

        mod.set_axon_ntff_profile_hook = _set
        antenv.axon_hooks = mod
        sys.modules["antenv.axon_hooks"] = mod
    except Exception:
        pass


def group_ext(g: int, causal: bool) -> int:
    """Causal rel extent (cols) for group g. Uniform across cores."""
    if not causal:
        return T
    if g < 16:
        return 64 * (g + 1)
    return 1024 + 64 * (g - 16 + 1)


def rel_chunks(g: int, causal: bool):
    """(offset, width) v-chunks for group g (width <= SCH)."""
    ext = group_ext(g, causal)
    out = []
    v0 = 0
    while v0 < ext:
        w = min(SCH, ext - v0)
        out.append((v0, w))
        v0 += w
    return out


def build_nc(causal: bool):
    exts = (1024, 2048) if causal else (2048, 2048)
    NST = T // 128            # v tiles per batch (16)

    nc = bacc.Bacc("TRN2", target_bir_lowering=False, debug=False,
                   num_devices=NCORES)

    # ---- I/O ----
    xh = nc.dram_tensor("xh", [D, T], F16, kind="ExternalInput")
    xl = nc.dram_tensor("xl", [D, T], F16, kind="ExternalInput")
    xqh = nc.dram_tensor("xqh", [D, 2, B, TB], F16, kind="ExternalInput")
    xql = nc.dram_tensor("xql", [D, 2, B, TB], F16, kind="ExternalInput")
    wkh = nc.dram_tensor("wkh", [D, H], F16, kind="ExternalInput")
    wkl = nc.dram_tensor("wkl", [D, H], F16, kind="ExternalInput")
    wqh = nc.dram_tensor("wqh", [D, H], F16, kind="ExternalInput")
    wql = nc.dram_tensor("wql", [D, H], F16, kind="ExternalInput")
    wv = nc.dram_tensor("wv", [D, H], F16, kind="ExternalInput")
    bk8 = nc.dram_tensor("bk8", [H, 1], F32, kind="ExternalInput")
    bq_ = nc.dram_tensor("bq", [H, 1], F32, kind="ExternalInput")
    # rel tiles, flattened stream: per (g, chunk) block [128, 4*w]
    totcols = sum(4 * w for g in range(NGRP) for _, w in rel_chunks(g, causal))
    relh = nc.dram_tensor("relh", [128, totcols], F16, kind="ExternalInput")
    maskrel = nc.dram_tensor("maskrel", [NGRP, 128, 64], F16,
                             kind="ExternalInput")
    identb = nc.dram_tensor("identb", [128, 128], F16, kind="ExternalInput")
    identf = nc.dram_tensor("identf", [64, 64], F16, kind="ExternalInput")
    out = nc.dram_tensor("out", [B, 2, TB, H], F32, kind="ExternalOutput")

    if DEBUG:
        dS0 = nc.dram_tensor("dS0", [TB, B * exts[0]], F16,
                             kind="ExternalOutput")
        dS1 = nc.dram_tensor("dS1", [TB, B * exts[1]], F16,
                             kind="ExternalOutput")
        dq16 = nc.dram_tensor("dq16", [64, 2 * B * TB], F16,
                              kind="ExternalOutput")
        dkst = nc.dram_tensor("dkst", [128, B * T], BF16,
                              kind="ExternalOutput")
        dvn = nc.dram_tensor("dvn", [128, B * (T // 128) * H], F16,
                             kind="ExternalOutput")
        dkgin = nc.dram_tensor("dkgin", [128, T], BF16,
                               kind="ExternalOutput")
        dkgout = nc.dram_tensor("dkgout", [NCORES, 128, T], BF16,
                                kind="ExternalOutput")
        dS2b0 = nc.dram_tensor("dS2b0", [TB, 1024], F32,
                               kind="ExternalOutput")
        dPb0 = nc.dram_tensor("dPb0", [TB, 1024], F16,
                              kind="ExternalOutput")
        dstat = nc.dram_tensor("dstat", [TB, 8], F32, kind="ExternalOutput")

    kg_in = nc.dram_tensor("kg_in", [128, T], BF16)
    kg_out = nc.dram_tensor("kg_out", [NCORES, 128, T], BF16,
                            addr_space="Shared")
    vg_in = nc.dram_tensor("vg_in", [128, NST * H], F16)
    vg_out = nc.dram_tensor("vg_out", [NCORES, 128, NST * H], F16,
                            addr_space="Shared")

    with tile.TileContext(nc) as tc:
        with (
            tc.tile_pool(name="persist", bufs=1) as pp,
            tc.tile_pool(name="weights", bufs=1) as pw,
            tc.tile_pool(name="Spool", bufs=1) as pS,
            tc.tile_pool(name="kv", bufs=1) as pkv,
            tc.tile_pool(name="relstream", bufs=4) as prel,
            tc.tile_pool(name="stg", bufs=3) as pstg,
            tc.tile_pool(name="stats", bufs=4) as pstat,
            tc.tile_pool(name="psrel", bufs=2, space="PSUM") as ppr,
        ):
            # persistent q tiles: cols (blk, b, t_local)
            qhi = pp.tile([64, 2 * B * TB], BF16, tag="qhi")
            qc = pp.tile([128, 2 * B * TB], BF16, tag="qc")   # [lo;hi]
            q16 = pp.tile([64, 2 * B * TB], F16, tag="q16")
            bd0 = pp.tile([128, (TB // 2) * 16], F16, tag="bd0")
            bd1 = pp.tile([128, (TB // 2) * 16], F16, tag="bd1")
            mrel = pp.tile([128, NGRP, 64], F16, tag="mrel")
            idb = pw.tile([128, 128], F16, tag="identb")
            idf = pw.tile([64, 64], F16, tag="identf")
            wk_t = pw.tile([128, ND, 2, H], F16, tag="wk")
            wq_t = pw.tile([128, ND, 2, H], F16, tag="wq")
            wv_t = pw.tile([128, ND, H], F16, tag="wv")
            bk_t = pw.tile([H, 1], F32, tag="bk")
            bq_t = pw.tile([H, 1], F32, tag="bq")

            nc.sync.dma_start(idb, identb.ap())
            nc.sync.dma_start(idf, identf.ap())
            nc.sync.dma_start(mrel, maskrel.ap().rearrange("g p v -> p g v"))
            nc.sync.dma_start(
                wk_t[:, :, 0, :], wkh.ap().rearrange("(n p) h -> p n h", p=128))
            nc.sync.dma_start(
                wk_t[:, :, 1, :], wkl.ap().rearrange("(n p) h -> p n h", p=128))
            nc.sync.dma_start(
                wq_t[:, :, 0, :], wqh.ap().rearrange("(n p) h -> p n h", p=128))
            nc.sync.dma_start(
                wq_t[:, :, 1, :], wql.ap().rearrange("(n p) h -> p n h", p=128))
            nc.sync.dma_start(
                wv_t, wv.ap().rearrange("(n p) h -> p n h", p=128))
            nc.sync.dma_start(bk_t, bk8.ap())
            nc.sync.dma_start(bq_t, bq_.ap())

            # rel+mask staging target: S tiles per block, fp16, init NEG
            S_all = [pS.tile([TB, B * exts[blk]], F16, tag=f"S{blk}",
                             name=f"S_{blk}")
                     for blk in range(2)]
            if causal:
                nc.vector.memset(S_all[0], NEG)
                nc.vector.memset(S_all[1], NEG)

            kstack = pkv.tile([128, B * T], BF16, tag="kstack")
            vnat = pkv.tile([128, B * NST * H], F16, tag="vnat")

            with ExitStack() as stk:
                ent = stk.enter_context
                pxh = ent(tc.tile_pool(name="xh", bufs=2))
                pxl = ent(tc.tile_pool(name="xl", bufs=2))
                pst = ent(tc.tile_pool(name="ptmp", bufs=2))
                ppush = ent(tc.tile_pool(name="push", bufs=2))
                ppmm = ent(tc.tile_pool(name="psproj", bufs=2, space="PSUM"))
                ppv = ent(tc.tile_pool(name="psv", bufs=1, space="PSUM"))
                ppvt = ent(tc.tile_pool(name="psvt", bufs=1, space="PSUM"))

                def kv_chunk(ci):
                    c0 = ci * SCHP
                    xf = xh.ap().rearrange("(n p) t -> p n t", p=128)
                    xlf = xl.ap().rearrange("(n p) t -> p n t", p=128)
                    xht = pxh.tile([128, ND, SCHP], F16, tag="xh")
                    xlt = pxl.tile([128, ND, SCHP], F16, tag="xl")
                    nc.sync.dma_start(xht, xf[:, :, c0:c0 + SCHP])
                    nc.sync.dma_start(xlt, xlf[:, :, c0:c0 + SCHP])
                    psk = ppmm.tile([H, SCHP], F32, tag="psp")
                    for dt_ in range(ND):
                        nc.tensor.matmul(psk, wk_t[:, dt_, 0, :], xht[:, dt_],
                                         start=(dt_ == 0), stop=False)
                    for dt_ in range(ND):
                        nc.tensor.matmul(psk, wk_t[:, dt_, 1, :], xht[:, dt_],
                                         start=False, stop=False)
                    for dt_ in range(ND):
                        nc.tensor.matmul(psk, wk_t[:, dt_, 0, :], xlt[:, dt_],
                                         start=False, stop=(dt_ == ND - 1))
                    ktmp = pst.tile([H, SCHP], F32, tag="ktmp")
                    nc.scalar.activation(ktmp, psk,
                                         mybir.ActivationFunctionType.Identity,
                                         bias=bk_t[:, :], scale=1.0)
                    kpush = ppush.tile([128, SCHP], BF16, tag="kpush")
                    nc.vector.tensor_copy(kpush[0:H, :], ktmp)
                    nc.vector.tensor_tensor(kpush[H:128, :], ktmp,
                                            kpush[0:H, :],
                                            mybir.AluOpType.subtract)
                    nc.sync.dma_start(kg_in.ap()[:, c0:c0 + SCHP], kpush)
                    psv = ppv.tile([H, SCHP], F32, tag="psv")
                    for dt_ in range(ND):
                        nc.tensor.matmul(psv, wv_t[:, dt_], xht[:, dt_],
                                         start=(dt_ == 0), stop=False)
                    for dt_ in range(ND):
                        nc.tensor.matmul(psv, wv_t[:, dt_], xlt[:, dt_],
                                         start=False, stop=(dt_ == ND - 1))
                    vtmp = pst.tile([H, SCHP], F16, tag="vtmp")
                    nc.scalar.copy(vtmp, psv)
                    vpush = ppush.tile([128, 2 * H], F16, tag="vpush")
                    for sub in range(SCHP // 128):
                        pvt = ppvt.tile([128, H], F16, tag="pvt")
                        nc.tensor.transpose(
                            pvt, vtmp[:, sub * 128:(sub + 1) * 128], idf)
                        nc.vector.tensor_copy(
                            vpush[:, sub * H:(sub + 1) * H], pvt)
                    st0 = c0 // 128
                    nc.sync.dma_start(
                        vg_in.ap()[:, st0 * H:(st0 + 2) * H], vpush)

                def q_chunk(ci):
                    c0 = ci * SCHP
                    xqf = xqh.ap().rearrange("(n p) k b t -> p n (k b t)",
                                             p=128)
                    xqlf = xql.ap().rearrange("(n p) k b t -> p n (k b t)",
                                              p=128)
                    xht = pxh.tile([128, ND, SCHP], F16, tag="xh")
                    xlt = pxl.tile([128, ND, SCHP], F16, tag="xl")
                    nc.sync.dma_start(xht, xqf[:, :, c0:c0 + SCHP])
                    nc.sync.dma_start(xlt, xqlf[:, :, c0:c0 + SCHP])
                    psq = ppmm.tile([H, SCHP], F32, tag="psp")
                    for dt_ in range(ND):
                        nc.tensor.matmul(psq, wq_t[:, dt_, 0, :], xht[:, dt_],
                                         start=(dt_ == 0), stop=False)
                    for dt_ in range(ND):
                        nc.tensor.matmul(psq, wq_t[:, dt_, 1, :], xht[:, dt_],
                                         start=False, stop=False)
                    for dt_ in range(ND):
                        nc.tensor.matmul(psq, wq_t[:, dt_, 0, :], xlt[:, dt_],
                                         start=False, stop=(dt_ == ND - 1))
                    qtmp = pst.tile([H, SCHP], F32, tag="ktmp")
                    nc.scalar.activation(qtmp, psq,
                                         mybir.ActivationFunctionType.Identity,
                                         bias=bq_t[:, :], scale=1.0)
                    cs = slice(c0, c0 + SCHP)
                    nc.vector.tensor_copy(qhi[:, cs], qtmp)
                    nc.vector.tensor_tensor(qc[0:H, cs], qtmp, qhi[:, cs],
                                            mybir.AluOpType.subtract)
                    nc.vector.tensor_copy(qc[H:128, cs], qhi[:, cs])
                    nc.vector.tensor_copy(q16[:, cs], qtmp)

                def build_bd(blk):
                    bdt = bd0 if blk == 0 else bd1
                    nc.vector.memset(bdt, 0.0)
                    qblk = (q16[:, blk * B * TB:(blk + 1) * B * TB]
                            .rearrange("c (b t) -> c b t", b=B))
                    for j in range(2):
                        dst = (bdt[j * H:(j + 1) * H]
                               .rearrange("c (p s) -> c p s", s=16)
                               [:, :, j * 8:j * 8 + 8])
                        src = qblk[:, :, j::2].rearrange("c b p -> c p b")
                        nc.vector.tensor_copy(dst, src)

                # ---- generators ----
                ccs = []
                gathered = {}

                def proj_steps():
                    # k/v projection first: its pushes feed the collectives,
                    # which must fire as early as possible to overlap the
                    # rel stream.
                    for ci in range(T // SCHP):
                        kv_chunk(ci)
                        yield
                    ccs.append(nc.gpsimd.collective_compute(
                        "AllGather", mybir.AluOpType.bypass,
                        replica_groups=[list(range(NCORES))],
                        ins=[kg_in[:].opt()], outs=[kg_out[:].opt()]))
                    ccs.append(nc.gpsimd.collective_compute(
                        "AllGather", mybir.AluOpType.bypass,
                        replica_groups=[list(range(NCORES))],
                        ins=[vg_in[:].opt()], outs=[vg_out[:].opt()]))
                    yield
                    for ci in range(T // SCHP):
                        q_chunk(ci)
                        if ci == 3:
                            build_bd(0)
                        yield
                    build_bd(1)
                    yield

                def rel_group(g, roff):
                    """Stream one rel group; returns new roff."""
                    blk = 0 if g < 16 else 1
                    gl = g - 16 * blk
                    ext = group_ext(g, causal)
                    bdt = bd0 if blk == 0 else bd1
                    stg = pstg.tile([128, exts[1]], F16, tag="stg",
                                    name=f"stg_{g}")
                    for (v0, w) in rel_chunks(g, causal):
                        rht = prel.tile([128, 4, SCH], F16, tag="rht")
                        nc.sync.dma_start(rht[:, :, 0:w],
                                          relh.ap()[:, roff:roff + 4 * w]
                                          .rearrange("p (u v) -> p u v", u=4))
                        roff += 4 * w
                        psr = ppr.tile([128, SCH], F32, tag="psr")
                        for u in range(4):
                            p = 4 * gl + u
                            nc.tensor.matmul(
                                psr[32 * u:32 * u + 16, 0:w],
                                bdt[:, p * 16:p * 16 + 16],
                                rht[:, u, 0:w],
                                start=True, stop=True,
                                tile_position=(0, 32 * u))
                        nc.scalar.copy(stg[:, v0:v0 + w],
                                              psr[:, 0:w])
                    if causal:
                        nc.gpsimd.tensor_tensor(
                            stg[:, ext - 64:ext], stg[:, ext - 64:ext],
                            mrel[:, g, :], mybir.AluOpType.add)
                    # scatter to S layout, one DMA per u-band:
                    # src [16 parts = (j,b), ext] -> dst [2 parts = j, (b, v)]
                    for u in range(4):
                        dst = (S_all[blk][8 * gl + 2 * u:8 * gl + 2 * u + 2, :]
                               .rearrange("j (b v) -> j b v", b=8)
                               [:, :, 0:ext])
                        src = stg[32 * u:32 * u + 16, 0:ext]
                        nc.gpsimd.dma_start(dst, src)
                    return roff

                def gather_kv():
                    # k/v from the AllGathers into SBUF (gpsimd SWDGE,
                    # spreads across all 16 SDMA engines). The collective's
                    # DRAM output is not dependency-tracked by Tile, so add
                    # explicit completion edges.
                    for b in range(B):
                        g1 = nc.gpsimd.dma_start(
                            kstack[:, b * T:(b + 1) * T], kg_out.ap()[b])
                        g2 = nc.gpsimd.dma_start(
                            vnat[:, b * NST * H:(b + 1) * NST * H],
                            vg_out.ap()[b])
                        gathered[b] = (g1, g2)
                        for gi in (g1, g2):
                            for cc in ccs:
                                bass._add_dep_helper(
                                    gi.ins, cc.ins, sync=True,
                                    reason="gather waits for AllGather")

                def rel_steps():
                    roff = 0
                    for g in range(NGRP):
                        roff = rel_group(g, roff)
                        if g == 19:
                            gather_kv()
                        yield g

                # drive: all kv chunks + collectives + q chunks 0-3 (bd0)
                # first, then interleave the rest of proj with rel tile0.
                pgen = proj_steps()
                rgen = rel_steps()
                done_p = False
                for _ in range(13):
                    done_p = next(pgen, "end") == "end"
                last_g = -1
                while last_g < 15:
                    if not done_p:
                        done_p = next(pgen, "end") == "end"
                    last_g = next(rgen)
                while not done_p:
                    done_p = next(pgen, "end") == "end"
                stk.close()

                # ---- qk/softmax/AV phase, interleaved with rel tile1 ----
                with ExitStack() as stk2:
                    ent2 = stk2.enter_context
                    pS2 = ent2(tc.tile_pool(name="S2", bufs=2))
                    pP = ent2(tc.tile_pool(name="P", bufs=2))
                    pPT = ent2(tc.tile_pool(name="PT", bufs=2))
                    po = ent2(tc.tile_pool(name="outp", bufs=10))
                    ppS = ent2(tc.tile_pool(name="psS", bufs=2, space="PSUM"))
                    pppt = ent2(tc.tile_pool(name="pspt", bufs=2,
                                             space="PSUM"))
                    ppav = ent2(tc.tile_pool(name="psav", bufs=1,
                                             space="PSUM"))

                    pending_out = []

                    def qk_b(blk, b, defer_out=False):
                        ext = exts[blk]
                        nch = ext // SCH
                        # during the rel-interleaved phase, keep the S2
                        # copies off the ACT engine (busy with rel copies)
                        ceng = nc.vector if defer_out else nc.scalar
                        S2 = pS2.tile([TB, exts[1]], F32, tag="S2")
                        pmax = pstat.tile([TB, 4], F32, tag="pmax")
                        qh_s = qhi[:, (blk * B + b) * TB:
                                   (blk * B + b + 1) * TB]
                        qc_s = qc[:, (blk * B + b) * TB:
                                  (blk * B + b + 1) * TB]
                        for ch in range(nch):
                            s0 = ch * SCH
                            psS = ppS.tile([TB, SCH], F32, tag="psS")
                            nc.tensor.matmul(
                                psS, idb,
                                S_all[blk][:, b * ext + s0:b * ext + s0 + SCH],
                                start=True, stop=False)
                            kc = slice(b * T + s0, b * T + s0 + SCH)
                            mmk = nc.tensor.matmul(psS, qh_s, kstack[0:H, kc],
                                                   start=False, stop=False)
                            nc.tensor.matmul(psS, qc_s, kstack[:, kc],
                                             start=False, stop=True)
                            if ch == 0 and b in gathered:
                                for gi in gathered[b]:
                                    bass._add_dep_helper(
                                        mmk.ins, gi.ins, sync=True,
                                        reason="qk waits for k/v gather")
                            if ceng is nc.vector:
                                nc.vector.tensor_copy(S2[:, s0:s0 + SCH], psS)
                            else:
                                nc.scalar.copy(S2[:, s0:s0 + SCH], psS)
                            nc.vector.tensor_reduce(
                                pmax[:, ch:ch + 1], psS,
                                mybir.AxisListType.X, mybir.AluOpType.max)
                        negmax = pstat.tile([TB, 1], F32, tag="negmax")
                        zsum = pstat.tile([TB, 1], F32, tag="zsum")
                        rz = pstat.tile([TB, 1], F32, tag="rz")
                        nc.vector.tensor_reduce(negmax, pmax[:, 0:nch],
                                                mybir.AxisListType.X,
                                                mybir.AluOpType.max,
                                                negate=True)
                        P = pP.tile([TB, exts[1]], F16, tag="P")
                        nc.scalar.activation(P[:, 0:ext], S2[:, 0:ext],
                                             mybir.ActivationFunctionType.Exp,
                                             bias=negmax[:, :], scale=1.0,
                                             accum_out=zsum[:, :])
                        nc.vector.reciprocal(rz, zsum)
                        pso = ppav.tile([TB, H], F32, tag="pso")
                        for st in range(ext // 128):
                            ppt = pppt.tile([128, 128], F16, tag="ppt")
                            nc.tensor.transpose(
                                ppt, P[:, st * 128:(st + 1) * 128], idb)
                            ptt = pPT.tile([128, 128], F16, tag="ptt")
                            nc.vector.tensor_copy(ptt, ppt)
                            nc.tensor.matmul(
                                pso, ptt,
                                vnat[:, (b * NST + st) * H:
                                     (b * NST + st + 1) * H],
                                start=(st == 0), stop=(st == ext // 128 - 1))
                        osb = po.tile([TB, H], F32, tag="osb",
                                      name=f"osb_{blk}_{b}")
                        nc.vector.tensor_scalar_mul(osb, pso, rz[:, :])
                        if defer_out:
                            # out-DMA on SP would stall the rel prefetch
                            # pipeline behind this qk chain; flush later.
                            pending_out.append((blk, b, osb))
                        else:
                            nc.sync.dma_start(out.ap()[b, blk], osb)
                        if DEBUG and blk == 0 and b == 0:
                            nc.sync.dma_start(dS2b0.ap(), S2[:, 0:1024])
                            nc.sync.dma_start(dPb0.ap(), P[:, 0:1024])
                            nc.sync.dma_start(dstat.ap()[:, 0:4], pmax)
                            nc.sync.dma_start(dstat.ap()[:, 4:5], negmax)
                            nc.sync.dma_start(dstat.ap()[:, 5:6], zsum)
                            nc.sync.dma_start(dstat.ap()[:, 6:7], rz)

                    # rel tile1 groups interleaved with qk blk0. The k/v
                    # gather is issued inside rel_steps after group 19, so
                    # qk consumers must only be issued after that point.
                    b0 = 0
                    while True:
                        g = next(rgen, None)
                        if g is None:
                            break
                        if g >= 21 and b0 < B:
                            qk_b(0, b0, defer_out=True)
                            b0 += 1
                    while b0 < B:
                        qk_b(0, b0, defer_out=True)
                        b0 += 1
                    for blk_, b_, osb_ in pending_out:
                        nc.sync.dma_start(out.ap()[b_, blk_], osb_)
                    pending_out.clear()
                    for b in range(B):
                        qk_b(1, b)

                    if DEBUG:
                        nc.sync.dma_start(dS0.ap(), S_all[0])
                        nc.sync.dma_start(dS1.ap(), S_all[1])
                        nc.sync.dma_start(dq16.ap(), q16)
                        nc.sync.dma_start(dkst.ap(), kstack)
                        nc.sync.dma_start(dvn.ap(), vnat)
                        nc.gpsimd.dma_start(dkgin.ap(), kg_in.ap())
                        nc.gpsimd.dma_start(dkgout.ap(), kg_out.ap())

    nc.compile()
    return nc


def _split16(a):
    hi = np.asarray(a, dtype=np.float32).astype(np.float16)
    lo = (np.asarray(a, dtype=np.float32) - hi.astype(np.float32)).astype(
        np.float16)
    return hi, lo


def _prep_core(c, xTh, xTl, rel16, causal):
    """Per-core input tensors (mod-8 row sharding)."""
    # q-projection x: cols (blk, b, t_local); t = blk*1024 + c + 8*j
    idx0 = c + 8 * np.arange(TB)
    idx1 = 1024 + c + 8 * np.arange(TB)
    xqh_c = np.stack([xTh[:, :, idx0], xTh[:, :, idx1]], axis=1)
    xql_c = np.stack([xTl[:, :, idx0], xTl[:, :, idx1]], axis=1)

    # rel stream: per (g, chunk) blocks [128, 4*w]
    blocks = []
    mtiles = np.zeros((NGRP, 128, 64), dtype=np.float16)
    for g in range(NGRP):
        blk, gl = (0, g) if g < 16 else (1, g - 16)
        base = blk * 1024
        trows = base + c + 8 * (8 * gl + np.arange(8))     # 8 query rows
        ext = group_ext(g, causal)
        A = rel16[trows, :ext, :]                          # [8, ext, 64]
        A = (A.reshape(4, 2, ext, 64).transpose(1, 3, 0, 2)
             .reshape(128, 4, ext))                        # [(j,c),(u,v)]
        for (v0, w) in rel_chunks(g, causal):
            blocks.append(np.ascontiguousarray(
                A[:, :, v0:v0 + w].reshape(128, 4 * w)))
        # mask tile for final 64 cols: partitions 32u+8j+b (x=0 half)
        if causal:
            for u in range(4):
                for j in range(2):
                    t = base + c + 8 * (8 * gl + 2 * u + j)
                    vv = ext - 64 + np.arange(64)
                    m = np.where(vv <= t, 0.0, NEG).astype(np.float16)
                    for b in range(8):
                        mtiles[g, 32 * u + 8 * j + b, :] = m
    relflat = np.concatenate(blocks, axis=1)
    return {
        "xh": np.ascontiguousarray(xTh[:, c, :]),
        "xl": np.ascontiguousarray(xTl[:, c, :]),
        "xqh": np.ascontiguousarray(xqh_c),
        "xql": np.ascontiguousarray(xql_c),
        "relh": relflat,
        "maskrel": mtiles,
    }


def kernel(x, Wk, bk, Wq, bq, Wv, rel_pos_emb, mask, **_unused):
    global LAST_EXEC_NS, LAST_RES
    x = np.asarray(x, dtype=np.float32)
    causal = bool(np.asarray(mask).item())

    scale = np.float32(np.sqrt(H))
    xT = np.ascontiguousarray(x.transpose(2, 0, 1))        # [D, B, T]
    xTh, xTl = _split16(xT)
    wkh, wkl = _split16(np.asarray(Wk, np.float32) * scale)
    wqh, wql = _split16(np.asarray(Wq, np.float32))
    wv16 = np.asarray(Wv, np.float32).astype(np.float16)
    bk8 = (np.asarray(bk, np.float32) * scale).reshape(H, 1)
    bqr = np.asarray(bq, np.float32).reshape(H, 1)
    rel16 = np.asarray(rel_pos_emb, np.float32).astype(np.float16)
    identb = np.eye(128, dtype=np.float16)
    identf = np.eye(64, dtype=np.float16)

    in_maps = []
    for c in range(NCORES):
        m = _prep_core(c, xTh, xTl, rel16, causal)
        m.update({
            "wkh": wkh, "wkl": wkl, "wqh": wqh, "wql": wql, "wv": wv16,
            "bk8": bk8, "bq": bqr, "identb": identb, "identf": identf,
        })
        in_maps.append(m)

    nc = build_nc(causal)
    if os.environ.get("KERNEL_TRACE") == "1":
        _install_ntff_hook()
        import jax
        jax.devices()
        try:
            res = run_bass_kernel_spmd(
                nc, in_maps, core_ids=list(range(NCORES)), trace=True)
        except Exception:
            res = run_bass_kernel_spmd(
                nc, in_maps, core_ids=list(range(NCORES)))
    else:
        res = run_bass_kernel_spmd(nc, in_maps, core_ids=list(range(NCORES)))
    LAST_EXEC_NS = res.exec_time_ns
    LAST_RES = res

    outf = np.empty((B, T, H), dtype=np.float32)
    for c in range(NCORES):
        oc = res.results[c]["out"]          # [B, 2, TB, H]
        idx0 = c + 8 * np.arange(TB)
        idx1 = 1024 + c + 8 * np.arange(TB)
        outf[:, idx0] = oc[:, 0]
        outf[:, idx1] = oc[:, 1]
    return outf
